# revision 1
# baseline (speedup 1.0000x reference)
"""Trainium2 Bass kernel for nn_Attention_60739427500161.

Strategy (8 NeuronCores, one chip, no collectives):
- Sequence-sharded (context parallel): core c handles batch b=c//4 and two
  zigzag 256-row query strips (ci*256 and (7-ci)*256, ci=c%4) so causal work
  is balanced. Each core computes q/gate for its 512 rows, the full-batch k
  projection locally (cheaper than an AllGather: the collective's entry
  barrier + latency + the power-governor clamp it triggered cost more than
  the extra 99k PE cycles), runs attention + gating + out_proj for its rows,
  and writes its 512 output rows. The host scatters them back.
- All matmuls run in bf16 (PE 1 cycle/row at full clock; f32/f32r operands
  are SBUF-bandwidth-bound at half rate). rel err ~1.05e-2 vs the f32
  reference, dominated by bf16 rounding of x/wq/wk/wo and q/k storage.
- Scores are computed transposed (scoresT[j,i]) so softmax needs no PE
  transposes: exp on ACT (no max-subtraction needed; causal mask -1e9
  becomes exp->0 via gpsimd affine_select), denominator l via a ones-column
  in the av matmul, and gating computes av / (l * (1 + e^{-g})) with one
  fused scalar_tensor_tensor + reciprocal_approx_fast (the gate exp shares
  the ACT Exp table with the attention exp - no table reloads).
- RoPE rotate_half is a host-side feature permutation (pairs (d, d+32)
  interleaved adjacently) so the partner lives one partition away and a DVE
  stream_shuffle([o^1]) produces the rotated operand.
- Per-core causal chunk counts differ, so the attention section is a 4-way
  tc.Switch on partition_id % 4. Inside each arm the qg projection tiles are
  interleaved with the attention heads as PE filler work so the TensorEngine
  never stalls on the ACT exp pipeline and the ACT duty cycle stays spread
  (high ACT duty while PE streams triggered a ~50% PE utilization clamp).
- Strip-merged scores/av: key chunks valid for both strips run as one
  N=512 matmul, halving LDWEIGHTS traffic for the shared prefix.
"""

import sys

for _p in ("/root/.axon_site/_ro/trn_rl_repo", "/opt/trn_rl_repo"):
    if _p not in sys.path:
        sys.path.append(_p)

import ml_dtypes
import numpy as np

import concourse.bass as bass
import concourse.mybir as mybir
import concourse.tile as tile
from concourse import bacc
from concourse.bass_utils import run_bass_kernel_spmd
from concourse.masks import make_identity
from concourse.tile_rust import add_dep_helper

F32 = mybir.dt.float32
F32R = mybir.dt.float32r
BF16 = mybir.dt.bfloat16
AF = mybir.ActivationFunctionType
ALU = mybir.AluOpType

B, S, HID = 2, 2048, 2048
NH, NKV, D = 32, 8, 64

# pi permutation: interleave (d, d+32) pairs so rotate_half partner is the
# adjacent partition. pos(d) = 2d (d<32) else 2(d-32)+1.
_POS = np.array([2 * d if d < 32 else 2 * (d - 32) + 1 for d in range(D)])
_INV = np.argsort(_POS)  # inv(p): even -> p//2, odd -> 32 + p//2
_SHUF_MASK = [o ^ 1 for o in range(32)]

# q-head placement: head h must share its SBUF row base (0 or 64) with its
# kv head hk=h//4 (kv pair layout fixes hk%2). Tile t pairs one even-hk head
# (rows 0-63) with one odd-hk head (rows 64-127).
_EVENS = [h for h in range(NH) if (h // 4) % 2 == 0]
_ODDS = [h for h in range(NH) if (h // 4) % 2 == 1]
_QTILE = [0] * NH
_QROW = [0] * NH
for _i, _h in enumerate(_EVENS):
    _QTILE[_h], _QROW[_h] = _i, 0
for _i, _h in enumerate(_ODDS):
    _QTILE[_h], _QROW[_h] = _i, 64
# source column order (within the 2048 q or gate cols) for host permutation
_QCOL_ORDER = np.concatenate(
    [np.concatenate([_EVENS[t] * D + _INV, _ODDS[t] * D + _INV]) for t in range(16)]
)

_NC_CACHE = None
DEBUG_DUMPS = False
N_PRELUDE = 10  # q-tiles emitted before the attention steps begin


def _build_nc():
    nc = bacc.Bacc(None, target_bir_lowering=False, enable_partition_id=True)

    xTq = nc.dram_tensor("xTq", [HID, 512], BF16, kind="ExternalInput")
    xTk = nc.dram_tensor("xTk", [HID, S], BF16, kind="ExternalInput")
    wqp = nc.dram_tensor("wqp", [32 * HID, 128], BF16, kind="ExternalInput")
    wkp = nc.dram_tensor("wkp", [HID, 512], BF16, kind="ExternalInput")
    wop = nc.dram_tensor("wop", [HID, HID], BF16, kind="ExternalInput")
    cosq = nc.dram_tensor("cosq", [128, 512], F32, kind="ExternalInput")
    sinq = nc.dram_tensor("sinq", [128, 512], F32, kind="ExternalInput")
    cosk = nc.dram_tensor("cosk", [128, 2048], F32, kind="ExternalInput")
    sink = nc.dram_tensor("sink", [128, 2048], F32, kind="ExternalInput")
    out = nc.dram_tensor("out", [512, HID], F32, kind="ExternalOutput")

    if DEBUG_DUMPS:
        dbg_qT = nc.dram_tensor("dbg_qT", [128, 8192], BF16, kind="ExternalOutput")
        dbg_sigT = nc.dram_tensor("dbg_sigT", [128, 8192], F32, kind="ExternalOutput")
        dbg_kT = nc.dram_tensor("dbg_kT", [128, 8192], BF16, kind="ExternalOutput")
        dbg_kaug = nc.dram_tensor("dbg_kaug", [128, NKV * 16 * 65], BF16, kind="ExternalOutput")
        dbg_gatedT = nc.dram_tensor("dbg_gatedT", [128, 8192], BF16, kind="ExternalOutput")

    with tile.TileContext(nc) as tc:
        ci = nc.partition_id() % 4

        with tc.tile_pool(name="persist", bufs=1) as pers:
            qT = pers.tile([128, 16 * 512], BF16, tag="qT")
            sigT = pers.tile([128, 16 * 512], F32, tag="sigT")
            kT = pers.tile([128, 4 * 2048], BF16, tag="kT")
            kaug = pers.tile([128, NKV * 16 * 65], BF16, tag="kaug")
            kaug4 = kaug[:].rearrange("p (h j d) -> p h j d", h=NKV, j=16)

            ident = pers.tile([128, 64], BF16, tag="ident")
            make_identity(nc, ident[0:64, :])
            nc.sync.dma_start(ident[64:128, :], ident[0:64, :])

            def emit_transposes(hk, PS_pool):
                """kaug chunks for kv head hk from the RoPE'd kT."""
                hkr = (hk % 2) * 64
                base = (hk // 2) * 2048
                for jj in range(4):
                    tr = PS_pool.tile([128, 256], BF16, tag="tr", bufs=2, name="tr")
                    for u in range(4):
                        jc = jj * 4 + u
                        nc.tensor.transpose(
                            tr[:, u * 64 : (u + 1) * 64],
                            kT[hkr : hkr + 64, base + jc * 128 : base + (jc + 1) * 128],
                            ident[hkr : hkr + 64, :],
                        )
                    nc.vector.tensor_copy(
                        kaug4[:, hk, jj * 4 : (jj + 1) * 4, 0:64],
                        tr[:].rearrange("p (u d) -> p u d", u=4),
                    )

            # ---- phase A: full-batch k projection (kT layout) + RoPE ----
            # Computed locally on every core (no collective: the entry
            # barrier + AllGather latency cost more than the extra 99k PE
            # cycles, and this removes cross-core skew).
            with (
                tc.tile_pool(name="pAtab", bufs=1) as pAtab,
                tc.tile_pool(name="pA", bufs=1) as pA,
                tc.tile_pool(name="pAr", bufs=2) as pAr,
                tc.tile_pool(name="psA", bufs=1, space="PSUM") as psA,
            ):
                wk_all = pA.tile([128, 16 * 512], BF16, tag="wk")
                xk_all = pA.tile([128, 16 * 2048], BF16, tag="xk")
                for kh in range(16):
                    nc.sync.dma_start(
                        wk_all[:, kh * 512 : (kh + 1) * 512],
                        wkp[kh * 128 : (kh + 1) * 128, :],
                    )
                    nc.sync.dma_start(
                        xk_all[:, kh * 2048 : (kh + 1) * 2048],
                        xTk[kh * 128 : (kh + 1) * 128, :],
                    )
                cosk_sb = pAtab.tile([128, 2048], F32, tag="cosk")
                sink_sb = pAtab.tile([128, 2048], F32, tag="sink")
                nc.sync.dma_start(cosk_sb[:], cosk[:])
                nc.sync.dma_start(sink_sb[:], sink[:])
                for tf in range(4):
                    for kb in range(4):
                        kp_ps = psA.tile([128, 512], F32, tag="kp", bufs=2, name="kp_ps")
                        for kc in range(16):
                            nc.tensor.matmul(
                                kp_ps[:],
                                wk_all[:, kc * 512 + tf * 128 : kc * 512 + (tf + 1) * 128],
                                xk_all[:, kc * 2048 + kb * 512 : kc * 2048 + (kb + 1) * 512],
                                start=(kc == 0),
                                stop=(kc == 15),
                            )
                        shf = pAr.tile([128, 512], F32, tag="shf")
                        nc.vector.stream_shuffle(shf[:], kp_ps[:], _SHUF_MASK)
                        t1 = pAr.tile([128, 512], F32, tag="t1")
                        nc.vector.tensor_tensor(
                            t1[:], kp_ps[:], cosk_sb[:, kb * 512 : (kb + 1) * 512], ALU.mult
                        )
                        t2 = pAr.tile([128, 512], F32, tag="t2")
                        nc.vector.tensor_tensor(
                            t2[:], shf[:], sink_sb[:, kb * 512 : (kb + 1) * 512], ALU.mult
                        )
                        nc.vector.tensor_tensor(
                            kT[:, tf * 2048 + kb * 512 : tf * 2048 + (kb + 1) * 512],
                            t1[:],
                            t2[:],
                            ALU.add,
                        )
                    if tf >= 1:
                        emit_transposes(2 * (tf - 1), psA)
                        emit_transposes(2 * (tf - 1) + 1, psA)
                emit_transposes(6, psA)
                emit_transposes(7, psA)
            nc.vector.memset(kaug4[:, :, :, 64:65], 1.0)

            with tc.tile_pool(name="pG", bufs=1) as pG:
              gatedT = pG.tile([128, 16 * 512], BF16, tag="gatedT")
              with (
                tc.tile_pool(name="pXq", bufs=1) as pXq,
                tc.tile_pool(name="pWq", bufs=5) as pWq,
                tc.tile_pool(name="pRt", bufs=2) as pRt,
                tc.tile_pool(name="pET", bufs=6) as pET,
                tc.tile_pool(name="pSm", bufs=3) as pSm,
                tc.tile_pool(name="PS", bufs=1, space="PSUM") as PS,
              ):
                xq = pXq.tile([128, 16 * 512], BF16, tag="xq")
                nc.sync.dma_start(
                    xq[:].rearrange("p (kc i) -> p kc i", kc=16),
                    xTq[:].rearrange("(kc p) i -> p kc i", p=128),
                )
                cosq_sb = pXq.tile([128, 512], F32, tag="cosq")
                sinq_sb = pXq.tile([128, 512], F32, tag="sinq")
                nc.sync.dma_start(cosq_sb[:], cosq[:])
                nc.sync.dma_start(sinq_sb[:], sinq[:])

                def emit_qg_tile(t):
                    """qg projection m-tile t (q-tile if t<16 else gate)."""
                    wq_t = pWq.tile([128, 16 * 128], BF16, tag="wq", name="wq_t")
                    nc.sync.dma_start(
                        wq_t[:].rearrange("p (kc m) -> p kc m", kc=16),
                        wqp[t * HID : (t + 1) * HID, :].rearrange(
                            "(kc p) m -> p kc m", p=128
                        ),
                    )
                    qg_ps = PS.tile([128, 512], F32, tag="qg", bufs=1, name="qg_ps")
                    for kc in range(16):
                        mm = nc.tensor.matmul(
                            qg_ps[:],
                            wq_t[:, kc * 128 : (kc + 1) * 128],
                            xq[:, kc * 512 : (kc + 1) * 512],
                            start=(kc == 0),
                            stop=(kc == 15),
                        )
                    if t < 16:
                        shf = pRt.tile([128, 512], F32, tag="shf", name="shf")
                        nc.vector.stream_shuffle(shf[:], qg_ps[:], _SHUF_MASK)
                        t1 = pRt.tile([128, 512], F32, tag="t1", name="t1")
                        nc.vector.tensor_tensor(t1[:], qg_ps[:], cosq_sb[:], ALU.mult)
                        t2 = pRt.tile([128, 512], F32, tag="t2", name="t2")
                        nc.vector.tensor_tensor(t2[:], shf[:], sinq_sb[:], ALU.mult)
                        nc.vector.tensor_tensor(
                            qT[:, t * 512 : (t + 1) * 512], t1[:], t2[:], ALU.add
                        )
                    else:
                        # e^{-g}; 1/(1+e^{-g}) is folded into the gating recip
                        nc.scalar.activation(
                            sigT[:, (t - 16) * 512 : (t - 15) * 512],
                            qg_ps[:],
                            AF.Exp,
                            scale=-1.0,
                        )
                    return mm

                def emit_attention_head(h, nja, njb):
                    """scoresT/exp/mask/av for head h; returns av psum tile."""
                    tq, hr, hk = _QTILE[h], _QROW[h], h // 4
                    kbase = (hk // 2) * 2048
                    av_ps = PS.tile([65, 512], F32, tag="av", bufs=3, name="av_ps")
                    rhs_m = qT[hr : hr + 64, tq * 512 : (tq + 1) * 512]
                    rhs_s = qT[hr : hr + 64, tq * 512 + 256 : (tq + 1) * 512]
                    groups = []
                    for g0 in range(0, nja, 2):
                        groups.append((range(g0, min(g0 + 2, nja)), 512))
                    for g0 in range(nja, njb, 4):
                        groups.append((range(g0, min(g0 + 4, njb)), 256))
                    pend = []  # (jcs, w, et) awaiting av emission

                    def flush_av():
                        jcs, w, et = pend.pop(0)
                        for u, jc in enumerate(jcs):
                            nc.tensor.matmul(
                                av_ps[0:65, (0 if w == 512 else 256) : 512],
                                kaug[:, (hk * 16 + jc) * 65 : (hk * 16 + jc) * 65 + 65],
                                et[:, u * w : u * w + w],
                                start=(jc == 0),
                                stop=(jc == njb - 1),
                                skip_group_check=True,
                            )

                    for jcs, w in groups:
                        sc = PS.tile([128, 1024], F32, tag="sc", bufs=2, name="sc")
                        et = pET.tile([128, 1024], BF16, tag="et", name="et")
                        for u, jc in enumerate(jcs):
                            nc.tensor.matmul(
                                sc[:, u * w : u * w + w],
                                kT[hr : hr + 64, kbase + jc * 128 : kbase + (jc + 1) * 128],
                                rhs_m if w == 512 else rhs_s,
                                start=True,
                                stop=True,
                            )
                        n_tot = len(jcs) * w
                        nc.scalar.activation(et[:, 0:n_tot], sc[:, 0:n_tot], AF.Exp)
                        for u, jc in enumerate(jcs):
                            if jc >= nja - 2 and jc < nja and w == 512:
                                nc.gpsimd.affine_select(
                                    et[:, u * w : u * w + 256],
                                    et[:, u * w : u * w + 256],
                                    pattern=[[1, 256]],
                                    compare_op=ALU.is_ge,
                                    fill=0.0,
                                    base=(0 if jc == nja - 2 else -128),
                                    channel_multiplier=-1,
                                )
                            if jc >= njb - 2:
                                off = u * w + (256 if w == 512 else 0)
                                nc.gpsimd.affine_select(
                                    et[:, off : off + 256],
                                    et[:, off : off + 256],
                                    pattern=[[1, 256]],
                                    compare_op=ALU.is_ge,
                                    fill=0.0,
                                    base=(0 if jc == njb - 2 else -128),
                                    channel_multiplier=-1,
                                )
                        pend.append((jcs, w, et))
                        if len(pend) > 1:
                            flush_av()
                    while pend:
                        flush_av()
                    return av_ps

                def emit_gating(h, av_ps):
                    # gated = av / (l * (1 + e^{-g})) ; l broadcast over the
                    # 64 feature partitions via gpsimd, one fused (eg+1)*lb,
                    # one approx reciprocal, one multiply.
                    tq, hr = _QTILE[h], _QROW[h]
                    lrow = pSm.tile([1, 512], F32, tag="lrow", name="lrow")
                    nc.scalar.copy(lrow[:], av_ps[64:65, :])
                    lb = pSm.tile([64, 512], F32, tag="lb", name="lb")
                    nc.gpsimd.partition_broadcast(lb[:], lrow[:])
                    eg = sigT[hr : hr + 64, tq * 512 : (tq + 1) * 512]
                    if hr:
                        egc = pSm.tile([64, 512], F32, tag="egc", name="egc")
                        nc.vector.tensor_copy(egc[:], eg)
                        eg = egc[:]
                    den = pSm.tile([64, 512], F32, tag="den", name="den")
                    nc.vector.scalar_tensor_tensor(
                        den[:], eg, 1.0, lb[:], ALU.add, ALU.mult
                    )
                    rden = pSm.tile([64, 512], F32, tag="rden", name="rden")
                    nc.vector.reciprocal_approx_fast(rden[:], den[:])
                    nc.vector.tensor_tensor(
                        gatedT[hr : hr + 64, tq * 512 : (tq + 1) * 512],
                        av_ps[0:64, :],
                        rden[:],
                        ALU.mult,
                    )

                for arm in tc.Switch(ci, 4):
                    nja, njb = 2 * arm + 2, 16 - 2 * arm
                    qfill = list(range(N_PRELUDE, 16))
                    deferred = []
                    for t in range(N_PRELUDE):
                        emit_qg_tile(t)
                    for t in range(16):
                        emit_qg_tile(16 + t)  # this step's gate tile
                        for h, av in deferred:
                            emit_gating(h, av)
                        deferred = []
                        av_a = emit_attention_head(_EVENS[t], nja, njb)
                        if qfill:
                            emit_qg_tile(qfill.pop(0))
                        av_b = emit_attention_head(_ODDS[t], nja, njb)
                        deferred = [(_EVENS[t], av_a), (_ODDS[t], av_b)]
                    for h, av in deferred:
                        emit_gating(h, av)

                if DEBUG_DUMPS:
                    nc.sync.dma_start(dbg_qT[:], qT[:])
                    nc.sync.dma_start(dbg_sigT[:], sigT[:])
                    nc.sync.dma_start(dbg_kT[:], kT[:])
                    nc.sync.dma_start(dbg_kaug[:], kaug[:])
                    nc.sync.dma_start(dbg_gatedT[:], gatedT[:])

              # ---- out projection (attention pools closed; pG alive) ----
              # wo chunks for one oc column stay resident (bufs=16) so the
              # mi-major loop reuses them and each mi's drain overlaps the
              # next mi's matmuls.
              with (
                  tc.tile_pool(name="pO", bufs=3) as pO,
                  tc.tile_pool(name="psO", bufs=1, space="PSUM") as psO,
              ):
                  for oc in range(4):
                      wo_ts = []
                      for fc in range(16):
                          wo_t = pO.tile(
                              [128, 512], BF16, tag="wo", bufs=18, name="wo_t"
                          )
                          nc.sync.dma_start(
                              wo_t[:],
                              wop[fc * 128 : (fc + 1) * 128, oc * 512 : (oc + 1) * 512],
                          )
                          wo_ts.append(wo_t)
                      for mi in range(4):
                          op_ps = psO.tile(
                              [128, 512], F32, tag="op", bufs=3, name="op_ps"
                          )
                          for fc in range(16):
                              nc.tensor.matmul(
                                  op_ps[:],
                                  gatedT[:, fc * 512 + mi * 128 : fc * 512 + (mi + 1) * 128],
                                  wo_ts[fc][:],
                                  start=(fc == 0),
                                  stop=(fc == 15),
                              )
                          o_sb = pO.tile([128, 512], F32, tag="ob", bufs=3, name="o_sb")
                          nc.scalar.copy(o_sb[:], op_ps[:])
                          nc.sync.dma_start(
                              out[mi * 128 : (mi + 1) * 128, oc * 512 : (oc + 1) * 512],
                              o_sb[:],
                          )
    nc.compile()
    return nc


def _get_nc():
    global _NC_CACHE
    if _NC_CACHE is None:
        _NC_CACHE = _build_nc()
    return _NC_CACHE


def _prep_inputs(hidden_states, cos, sin, wq, wk, wo):
    """Build the 8 per-core input maps (all host-side slicing/permutation)."""
    inv = _INV
    dmap = np.concatenate([inv, inv])  # d index for partition p (p%64)
    sign = np.where((np.arange(128) % 64) % 2 == 0, -1.0, 1.0).astype(np.float32)

    wq_q = wq[:, :2048][:, _QCOL_ORDER]
    wq_g = wq[:, 2048:][:, _QCOL_ORDER]
    wqp_flat = np.concatenate([wq_q, wq_g], axis=1)  # [HID, 4096]
    # pre-tile: [t, kc, p, m] contiguous so each [128,128] lhsT chunk is one
    # dense block in HBM (avoids read amplification on 256B segments)
    wqp = np.ascontiguousarray(
        wqp_flat.reshape(16, 128, 32, 128).transpose(2, 0, 1, 3).reshape(32 * HID, 128)
    ).astype(ml_dtypes.bfloat16)
    wkp = np.ascontiguousarray(
        wk.reshape(HID, NKV, D)[:, :, inv].reshape(HID, 512)
    ).astype(ml_dtypes.bfloat16)
    wop = np.ascontiguousarray(wo[_QCOL_ORDER, :]).astype(ml_dtypes.bfloat16)

    in_maps = []
    for c in range(8):
        b, cc = c // 4, c % 4
        r0a, r0b = cc * 256, (7 - cc) * 256
        qrows = np.concatenate([np.arange(r0a, r0a + 256), np.arange(r0b, r0b + 256)])
        xT = hidden_states[b].T  # [HID, S]
        cq = cos[qrows][:, dmap].T  # [128, 512]
        sq = (sin[qrows][:, dmap].T * sign[:, None]).astype(np.float32)
        ck = cos[:, dmap].T  # [128, 2048] all key positions
        sk = (sin[:, dmap].T * sign[:, None]).astype(np.float32)
        in_maps.append(
            {
                "xTq": np.ascontiguousarray(xT[:, qrows]).astype(ml_dtypes.bfloat16),
                "xTk": np.ascontiguousarray(xT).astype(ml_dtypes.bfloat16),
                "wqp": wqp,
                "wkp": wkp,
                "wop": wop,
                "cosq": np.ascontiguousarray(cq),
                "sinq": np.ascontiguousarray(sq),
                "cosk": np.ascontiguousarray(ck),
                "sink": np.ascontiguousarray(sk),
            }
        )
    return in_maps


def kernel(hidden_states, cos, sin, attention_mask, wq, wk, wv, wo, **_unused):
    hidden_states = np.asarray(hidden_states, dtype=np.float32)
    cos = np.asarray(cos, dtype=np.float32)
    sin = np.asarray(sin, dtype=np.float32)
    wq = np.asarray(wq, dtype=np.float32)
    wk = np.asarray(wk, dtype=np.float32)
    wo = np.asarray(wo, dtype=np.float32)

    nc = _get_nc()
    in_maps = _prep_inputs(hidden_states, cos, sin, wq, wk, wo)
    res = run_bass_kernel_spmd(nc, in_maps, core_ids=list(range(8)))

    y = np.empty((B, S, HID), dtype=np.float32)
    for c in range(8):
        b, cc = c // 4, c % 4
        r0a, r0b = cc * 256, (7 - cc) * 256
        o = res.results[c]["out"]
        y[b, r0a : r0a + 256] = o[0:256]
        y[b, r0b : r0b + 256] = o[256:512]
    return y

